# revision 35
# baseline (speedup 1.0000x reference)
"""MultiHeadAttention Trainium2 kernel.

Full inputs: x [4, 2048, 768] f32, W_qkv [2304, 768], W_proj [768, 768],
b_proj [768]. Output [4, 2048, 768] f32.

Sharding: 8 cores = 4 batches x 2 head-groups (6 heads each).
Per-core inputs (host-prepared, transposed on host):
  xT  [768, 2048]  = x[b].T
  wT  [768, 1152]  = concat(Wq_g, Wk_g, Wv_g).T   (g = head group rows)
  wpT [384, 768]   = W_proj[:, g-cols].T
Per-core output: outp [2048, 768] = partial projection output for batch b.
Host: out[b] = outp[2b] + outp[2b+1] + b_proj.

Key optimizations vs the 332.9us baseline (275us -> ~258us this session):
  - QK matmuls run as row-tiled pairs (tile_position inferred from base
    partitions 0:64 / 64:128): two K=64 matmuls execute concurrently in
    the PE array, halving QK cost. kT_sb stores head pairs like qT_sb.
  - The softmax exp is split between the Scalar engine (real Exp LUT,
    odd kk chunks) and the Vector engine (even chunks; Schraudolph
    bit-trick: y = E*(128/ln2/8) + bias + 2^23 in f32 — the low 16 bits
    of y's mantissa ARE the bf16 of e^E; the AV matmul reads them as a
    stride-2 bf16 view). Both engines are PSUM-read-port bound (ACT
    (172+1024)/1.2 = 997ns/chunk, DVE (120+1024)/0.96 = 1192ns), so
    phase 2 is exp-engine bound at ~12us/unit. NOTE: the strict 8:8
    odd/even alternation is load-bearing — any cadence change (9:7
    split, per-chunk column offload, split-engine boundary chunks,
    cross-engine same-slot deps) collapses the pipeline by 10-50us.
  - The AV stationary carries the 64 v columns PLUS 64 ones columns, so
    av psum rows 64:128 hold the softmax denominator l replicated
    64-wide; reciprocal_approx_fast needs a base-partition-0 SBUF input
    (custom-DVE ops misbehave at base partition 64), so l is copied
    down first. attT multiplies run on GPSIMD (no PSUM port, so the av
    rows must be copied to SBUF for it).
  - Norm work is spread one op per 2 kk slots (kks 1..11) of the next
    unit so neither exp engine ever sees a burst that delays the
    ring-critical exps.
  - Phase 2 emits the AV pair BEFORE the QK pair in each kk slot: the
    AVs' deps are old, so they stream while the QK pair waits on the
    3-deep e2 psum ring (PSUM: 2 av accumulators + 3x2 energy tiles).
  - Units run n-outer; the output projection for query group g runs as
    8 half-blocks inserted into the AV-drain regions of group g+1's
    units (sharing the eps psum ring), so the output DMA overlaps
    phase-2 compute. Output partials are written as bf16 (halves the
    6.3MB->3.15MB output DMA; the host upcasts and sums).
  - Phase 1 runs c-outermost over groups of 2 q/k blocks (eps ring is
    3-deep, so group g+1 never waits on group g's drains), consuming
    each 820KB input chunk as its DMA lands instead of serializing on
    the last chunk; 2 v accumulators (in the phase-1-idle av psum tag)
    ride along per group, the rest run after the stream.
  - wp (phase-3 weights) DMA is deferred past the x/w input DMAs.
"""

import ml_dtypes
import numpy as np

import concourse.bass as bass
import concourse.tile as tile
from concourse import bacc, mybir
from concourse.bass_utils import run_bass_kernel_spmd

EMB = 768
N = 2048
B = 4
D = 64
HL = 6            # heads per core
HD = HL * D       # 384 local head-dim columns
NCORES = 8
SCALE = D ** -0.5

F32 = mybir.dt.float32
BF16 = mybir.dt.bfloat16
I16 = mybir.dt.int16

EC = EMB // 128   # 6 emb chunks
MC = HD // 128    # 3 head pairs
NQ = N // 512     # 4 query chunks of 512
NK = N // 128     # 16 key chunks of 128
DEPTH = 8         # AV software-pipeline depth (in kk steps)

EXP = mybir.ActivationFunctionType.Exp
MULT = mybir.AluOpType.mult
ADD = mybir.AluOpType.add

ASC = float(128.0 / np.log(2) * SCALE)      # schraudolph slope (scale folded)
BMAGIC = float(16250.5 + 2 ** 23)           # schraudolph bias + f32 round trick


def _emit(tc):
    from contextlib import ExitStack

    nc = tc.nc
    xT = nc.dram_tensor("xT", [EMB, N], BF16, kind="ExternalInput").ap()
    wT = nc.dram_tensor("wT", [EMB, 3 * HD], BF16, kind="ExternalInput").ap()
    wpT = nc.dram_tensor("wpT", [HD, EMB], BF16, kind="ExternalInput").ap()
    outp = nc.dram_tensor("outp", [N, EMB], BF16, kind="ExternalOutput").ap()

    xTr = xT.rearrange("(c p) s -> p c s", p=128)
    wTr = wT.rearrange("(c p) s -> p c s", p=128)
    wpTr = wpT.rearrange("(m p) e -> p m e", p=128)
    outr = outp.rearrange("(s p) e -> p s e", p=128)

    with ExitStack() as persist:
        ppool = persist.enter_context(tc.tile_pool(name="persist", bufs=1))
        # PE warmup: junk matmuls run during the input-DMA wait to open the
        # HAM clock gate
        warm_sb = ppool.tile([128, 640], BF16)
        nc.vector.memset(warm_sb[:], 1.0)
        wp_sb = ppool.tile([128, MC, EMB], BF16)
        qT_sb = ppool.tile([128, MC, N], BF16)
        kT_sb = ppool.tile([128, MC, N], BF16)
        # per head block: [v columns (64) | ones columns (64)] so the AV
        # matmul also produces l replicated across 64 psum rows
        v_sb = ppool.tile([128, NK, HL * 2 * D], BF16)
        nc.vector.memset(
            v_sb[:].rearrange("p k (h c) -> p k h c", c=2 * D)[:, :, :, D:2 * D],
            1.0)
        attT_sb = ppool.tile([128, MC, N], BF16)

        psum_pool = persist.enter_context(
            tc.tile_pool(name="psum", bufs=1, space="PSUM"))
        warm_ps = psum_pool.tile([128, 512], F32, tag="av", bufs=2, name="warm_ps")
        for wi in range(10):
            nc.tensor.matmul(warm_ps[:], warm_sb[:, 0:128], warm_sb[:, 128:640],
                             start=(wi == 0), stop=(wi == 9))

        # ---- phases 1+2+3 share a scope (x/w stay resident through
        # phase 1; o_sb staging lives through phase 2's interleaved
        # output projection) ----
        with ExitStack() as ph2:
            p1 = ph2.enter_context(tc.tile_pool(name="ph1", bufs=1))
            x_sb = p1.tile([128, EC, N], BF16)
            w_sb = p1.tile([128, EC, 3 * HD], BF16)
            for c in range(EC):
                nc.sync.dma_start(w_sb[:, c, :], wTr[:, c, :])
                nc.sync.dma_start(x_sb[:, c, :], xTr[:, c, :])
            # wp is only needed in phase 3; don't put it ahead of x/w
            nc.sync.dma_start(wp_sb[:], wpTr)

            # phase 1, c-outermost over groups of 3 q/k blocks (the eps
            # psum ring depth): every input chunk c is consumed by all
            # in-flight accumulators as soon as its DMA lands, so compute
            # tracks the ~13us input stream instead of serializing on the
            # last chunk. Two v accumulators ride along per group; the
            # remaining v tiles run after the stream (inputs resident).
            v_state = {}

            def v_step(s, c):
                if s not in v_state:
                    v_state[s] = psum_pool.tile([128, 512], F32, tag="av",
                                                bufs=2, name=f"vv_{s}")[:, 0:HD]
                nc.tensor.matmul(
                    v_state[s],
                    (x_sb[:, c, s * 128:(s + 1) * 128]),
                    (w_sb[:, c, 2 * HD:3 * HD]),
                    start=(c == 0), stop=(c == EC - 1))
                if c == EC - 1:
                    nc.vector.tensor_copy(
                        v_sb[:, s, :].rearrange(
                            "p (h c) -> p h c", c=2 * D)[:, :, 0:D],
                        v_state.pop(s)[:].rearrange("p (h d) -> p h d", h=HL))

            blocks = [(which, m, nh) for which in (0, 1)
                      for m in range(MC) for nh in (0, 1)]
            for g in range(6):
                group = blocks[2 * g:2 * g + 2]
                mm4s = {}
                for b, (which, m, nh) in enumerate(group):
                    mm4s[b] = psum_pool.tile([128, 2, 512], F32, tag="eps",
                                             bufs=3, name=f"mm4_{g}_{b}")
                for c in range(EC):
                    for b, (which, m, nh) in enumerate(group):
                        lo = which * HD + m * 128
                        for j in (0, 1):
                            n = 2 * nh + j
                            nc.tensor.matmul(
                                mm4s[b][:, j, :],
                                (w_sb[:, c, lo:lo + 128]),
                                (x_sb[:, c, n * 512:(n + 1) * 512]),
                                start=(c == 0), stop=(c == EC - 1))
                    v_step(2 * g, c)
                    v_step(2 * g + 1, c)
                for b, (which, m, nh) in enumerate(group):
                    dst = qT_sb if which == 0 else kT_sb
                    for j in (0, 1):
                        n = 2 * nh + j
                        ns = slice(n * 512, (n + 1) * 512)
                        if (which + n) % 2 == 0:
                            nc.scalar.copy(dst[:, m, ns], mm4s[b][:, j, :])
                        else:
                            nc.vector.tensor_copy(dst[:, m, ns], mm4s[b][:, j, :])
            for s0 in range(12, NK, 2):
                for c in range(EC):
                    v_step(s0, c)
                    v_step(s0 + 1, c)

            esb_pool = ph2.enter_context(tc.tile_pool(name="esb", bufs=4))
            sm_pool = ph2.enter_context(tc.tile_pool(name="sm", bufs=4))
            osb_pool = ph2.enter_context(tc.tile_pool(name="osb", bufs=3))

            def make_pr_jobs(n):
                # output projection for query group n: 4 s-chunks x 2
                # halves sharing the eps psum ring; DMA fires per s-chunk
                jobs = []
                for s in range(4 * n, 4 * n + 4):
                    o_sb = osb_pool.tile([128, EMB], BF16, tag="osb",
                                         name=f"osb_{s}")
                    for half in range(2):
                        def job(s=s, half=half, o_sb=o_sb):
                            pr = psum_pool.tile([128, 2, 512], F32, tag="eps",
                                                bufs=3,
                                                name=f"pr_{s}_{half}")[:, 0, 0:HD]
                            for mm in range(MC):
                                nc.tensor.matmul(
                                    pr[:],
                                    (attT_sb[:, mm, s * 128:(s + 1) * 128]),
                                    (wp_sb[:, mm, half * HD:(half + 1) * HD]),
                                    start=(mm == 0), stop=(mm == MC - 1))
                            if half == 0:
                                nc.vector.tensor_copy(o_sb[:, 0:HD], pr[:])
                            else:
                                nc.scalar.copy(o_sb[:, HD:2 * HD], pr[:])
                                nc.sync.dma_start(outr[:, s, :], o_sb[:])
                        jobs.append(job)
                return jobs

            pr_queue = []
            pending_norm = {}
            for n in range(NQ):
                for m in range(MC):
                    unit = n * MC + m
                    ns = slice(n * 512, (n + 1) * 512)
                    kslice = lambda kk: slice(kk * 128, (kk + 1) * 128)
                    av_t = [psum_pool.tile([128, 512], F32, tag="av", bufs=2,
                                           name=f"av_{m}_{n}_{z}")
                            for z in (0, 1)]
                    mvq = []

                    def emit_av(j):
                        for z in (0, 1):
                            h = 2 * m + z
                            nc.tensor.matmul(
                                av_t[z][:],
                                (v_sb[:, j, h * 2 * D:(h + 1) * 2 * D]),
                                mvq[j][z],
                                start=(j == 0), stop=(j == NK - 1))

                    for kk in range(NK):
                        if kk >= DEPTH:
                            emit_av(kk - DEPTH)
                        e2 = psum_pool.tile([128, 2, 512], F32, tag="eps",
                                            bufs=3, name=f"e_{m}_{n}_{kk}")
                        nc.tensor.matmul(e2[:, 0, :],
                                         (kT_sb[0:64, m, kslice(kk)]),
                                         (qT_sb[0:64, m, ns]),
                                         start=True, stop=True)
                        nc.tensor.matmul(e2[:, 1, :],
                                         (kT_sb[64:128, m, kslice(kk)]),
                                         (qT_sb[64:128, m, ns]),
                                         start=True, stop=True)
                        # exp FIRST (ring-critical), then the deferred
                        # norm ops: their deps are a unit old, but emitting
                        # them earlier would queue them ahead of the exp on
                        # the same engine FIFO and delay the e2 ring.
                        if kk % 2 == 1:
                            esb = esb_pool.tile([128, 2, 512], BF16, tag="esb",
                                                bufs=7, name=f"esb_{m}_{n}_{kk}")
                            nc.scalar.activation(esb[:], e2[:], EXP, scale=SCALE)
                            mvq.append((esb[:, 0, :], esb[:, 1, :]))
                        else:
                            esf = esb_pool.tile([128, 2, 512], F32, tag="esf",
                                                bufs=7, name=f"esf_{m}_{n}_{kk}")
                            nc.vector.tensor_scalar(esf[:], e2[:], ASC, BMAGIC,
                                                    MULT, ADD)
                            bv = esf[:].bitcast(I16)[:, :, 0::2].bitcast(BF16)
                            mvq.append((bv[:, 0, :], bv[:, 1, :]))
                        if kk in pending_norm:
                            for fn in pending_norm.pop(kk):
                                fn()
                    for j in range(NK - DEPTH, NK):
                        emit_av(j)
                        if j in (11, 13, 15) and pr_queue:
                            pr_queue.pop(0)()

                    # drain + normalize for this (pair, n):
                    # rows 0:64 = av, rows 64:128 = l replicated 64-wide.
                    # copies split 3 ACT / 1 DVE; 1/l on DVE; the attT
                    # multiplies run on GPSIMD (own queue, off the exp path).
                    # All ops are spread into the next unit's first kk slots
                    # so neither engine sees a boundary burst.
                    tiles = []
                    for z in (0, 1):
                        tiles.append((
                            sm_pool.tile([D, 512], F32, tag=f"avst{z}",
                                         bufs=3, name=f"avst_{m}_{n}_{z}"),
                            sm_pool.tile([D, 512], F32, tag=f"lrep{z}",
                                         bufs=3, name=f"lrep_{m}_{n}_{z}"),
                            sm_pool.tile([D, 512], F32, tag=f"rb{z}",
                                         bufs=3, name=f"rb_{m}_{n}_{z}")))

                    def norm_ops(m=m, ns=ns, av_t=av_t, tiles=tiles):
                        a0, l0, r0 = tiles[0]
                        a1, l1, r1 = tiles[1]

                        def mul(z, a, r):
                            nc.gpsimd.tensor_mul(
                                attT_sb[z * 64:(z + 1) * 64, m, ns], a[:], r[:])
                        return {
                            1: [lambda: nc.scalar.copy(a0[:], av_t[0][0:D, :])],
                            3: [lambda: nc.scalar.copy(a1[:], av_t[1][0:D, :])],
                            5: [lambda: nc.scalar.copy(
                                    l0[:], av_t[0][D:2 * D, :])],
                            7: [lambda: nc.scalar.copy(l1[:],
                                                       av_t[1][D:2 * D, :]),
                                lambda: nc.vector.reciprocal_approx_fast(
                                    r0[:], l0[:])],
                            9: [lambda: nc.vector.reciprocal_approx_fast(
                                    r1[:], l1[:]),
                                lambda: mul(0, a0, r0)],
                            11: [lambda: mul(1, a1, r1)],
                        }

                    if unit == MC * NQ - 1:
                        for kk, fns in sorted(norm_ops().items()):
                            for fn in fns:
                                fn()
                    else:
                        pending_norm = norm_ops()
                if n > 0:
                    pr_queue.extend(make_pr_jobs(n - 1))

            # keep the PE busy through the last unit's normalization, then
            # run the last group's output projection
            fill_ps = psum_pool.tile([128, 512], F32, tag="av", bufs=2,
                                     name="fill_ps")
            for wi in range(16):
                nc.tensor.matmul(fill_ps[:], warm_sb[:, 0:128],
                                 warm_sb[:, 128:640],
                                 start=(wi == 0), stop=(wi == 15))
            pr_queue.extend(make_pr_jobs(NQ - 1))
            while pr_queue:
                pr_queue.pop(0)()


_CACHE = {}


def _build():
    if "nc" not in _CACHE:
        nc = bacc.Bacc("TRN2", target_bir_lowering=False, debug=False,
                       num_devices=NCORES)
        with tile.TileContext(nc) as tc:
            _emit(tc)
        nc.compile()
        _CACHE["nc"] = nc
    return _CACHE["nc"]


def _in_maps(x, W_qkv, W_proj):
    in_maps = []
    for c in range(NCORES):
        b, g = divmod(c, 2)
        r0 = g * HD
        w_rows = np.concatenate([
            W_qkv[0 * EMB + r0: 0 * EMB + r0 + HD],
            W_qkv[1 * EMB + r0: 1 * EMB + r0 + HD],
            W_qkv[2 * EMB + r0: 2 * EMB + r0 + HD],
        ], axis=0)                                   # [1152, 768]
        bf = ml_dtypes.bfloat16
        in_maps.append({
            "xT": np.ascontiguousarray(x[b].T.astype(bf)),
            "wT": np.ascontiguousarray(w_rows.T.astype(bf)),
            "wpT": np.ascontiguousarray(W_proj[:, r0:r0 + HD].T.astype(bf)),
        })
    return in_maps


LAST_RESULTS = None


def kernel(x, W_qkv, W_proj, b_proj):
    global LAST_RESULTS
    x = np.ascontiguousarray(np.asarray(x, dtype=np.float32))
    W_qkv = np.asarray(W_qkv, dtype=np.float32)
    W_proj = np.asarray(W_proj, dtype=np.float32)
    b_proj = np.asarray(b_proj, dtype=np.float32)

    nc = _build()
    in_maps = _in_maps(x, W_qkv, W_proj)
    res = run_bass_kernel_spmd(nc, in_maps, core_ids=list(range(NCORES)))
    LAST_RESULTS = res

    out = np.empty((B, N, EMB), dtype=np.float32)
    for b in range(B):
        out[b] = (res.results[2 * b]["outp"].astype(np.float32)
                  + res.results[2 * b + 1]["outp"].astype(np.float32))
    out += b_proj
    return out



# revision 36
# speedup vs baseline: 1.2095x; 1.2095x over previous
"""MultiHeadAttention Trainium2 kernel.

Full inputs: x [4, 2048, 768] f32, W_qkv [2304, 768], W_proj [768, 768],
b_proj [768]. Output [4, 2048, 768] f32.

Sharding: 8 cores = 4 batches x 2 head-groups (6 heads each).
Per-core inputs (host-prepared, transposed on host):
  xT  [768, 2048]  = x[b].T
  wT  [768, 1152]  = concat(Wq_g, Wk_g, Wv_g).T   (g = head group rows)
  wpT [384, 768]   = W_proj[:, g-cols].T
Per-core output: outp [2048, 768] = partial projection output for batch b.
Host: out[b] = outp[2b] + outp[2b+1] + b_proj.

Key optimizations vs the 332.9us baseline (275us -> ~258us this session):
  - QK matmuls run as row-tiled pairs (tile_position inferred from base
    partitions 0:64 / 64:128): two K=64 matmuls execute concurrently in
    the PE array, halving QK cost. kT_sb stores head pairs like qT_sb.
  - The softmax exp is split between the Scalar engine (real Exp LUT,
    odd kk chunks) and the Vector engine (even chunks; Schraudolph
    bit-trick: y = E*(128/ln2/8) + bias + 2^23 in f32 — the low 16 bits
    of y's mantissa ARE the bf16 of e^E; the AV matmul reads them as a
    stride-2 bf16 view). Both engines are PSUM-read-port bound (ACT
    (172+1024)/1.2 = 997ns/chunk, DVE (120+1024)/0.96 = 1192ns), so
    phase 2 is exp-engine bound at ~12us/unit. NOTE: the strict 8:8
    odd/even alternation is load-bearing — any cadence change (9:7
    split, per-chunk column offload, split-engine boundary chunks,
    cross-engine same-slot deps) collapses the pipeline by 10-50us.
  - The AV stationary carries the 64 v columns PLUS 64 ones columns, so
    av psum rows 64:128 hold the softmax denominator l replicated
    64-wide; reciprocal_approx_fast needs a base-partition-0 SBUF input
    (custom-DVE ops misbehave at base partition 64), so l is copied
    down first. attT multiplies run on GPSIMD (no PSUM port, so the av
    rows must be copied to SBUF for it).
  - Norm work is spread one op per 2 kk slots (kks 1..11) of the next
    unit so neither exp engine ever sees a burst that delays the
    ring-critical exps.
  - Phase 2 emits the AV pair BEFORE the QK pair in each kk slot: the
    AVs' deps are old, so they stream while the QK pair waits on the
    3-deep e2 psum ring (PSUM: 2 av accumulators + 3x2 energy tiles).
  - Units run n-outer; the output projection for query group g runs as
    8 half-blocks inserted into the AV-drain regions of group g+1's
    units (sharing the eps psum ring), so the output DMA overlaps
    phase-2 compute. Output partials are written as bf16 (halves the
    6.3MB->3.15MB output DMA; the host upcasts and sums).
  - Phase 1 runs c-outermost over groups of 2 q/k blocks (eps ring is
    3-deep, so group g+1 never waits on group g's drains), consuming
    each 820KB input chunk as its DMA lands instead of serializing on
    the last chunk; 2 v accumulators (in the phase-1-idle av psum tag)
    ride along per group, the rest run after the stream.
  - wp (phase-3 weights) DMA is deferred past the x/w input DMAs.
"""

import ml_dtypes
import numpy as np

import concourse.bass as bass
import concourse.tile as tile
from concourse import bacc, mybir
from concourse.bass_utils import run_bass_kernel_spmd

EMB = 768
N = 2048
B = 4
D = 64
HL = 6            # heads per core
HD = HL * D       # 384 local head-dim columns
NCORES = 8
SCALE = D ** -0.5

F32 = mybir.dt.float32
BF16 = mybir.dt.bfloat16
I16 = mybir.dt.int16

EC = EMB // 128   # 6 emb chunks
MC = HD // 128    # 3 head pairs
NQ = N // 512     # 4 query chunks of 512
NK = N // 128     # 16 key chunks of 128
DEPTH = 8         # AV software-pipeline depth (in kk steps)

EXP = mybir.ActivationFunctionType.Exp
MULT = mybir.AluOpType.mult
ADD = mybir.AluOpType.add

ASC = float(128.0 / np.log(2) * SCALE)      # schraudolph slope (scale folded)
BMAGIC = float(16250.5 + 2 ** 23)           # schraudolph bias + f32 round trick


def _emit(tc):
    from contextlib import ExitStack

    nc = tc.nc
    xT = nc.dram_tensor("xT", [EMB, N], BF16, kind="ExternalInput").ap()
    wT = nc.dram_tensor("wT", [EMB, 3 * HD], BF16, kind="ExternalInput").ap()
    wpT = nc.dram_tensor("wpT", [HD, EMB], BF16, kind="ExternalInput").ap()
    outp = nc.dram_tensor("outp", [N, EMB], BF16, kind="ExternalOutput").ap()

    xTr = xT.rearrange("(c p) s -> p c s", p=128)
    wTr = wT.rearrange("(c p) s -> p c s", p=128)
    wpTr = wpT.rearrange("(m p) e -> p m e", p=128)
    outr = outp.rearrange("(s p) e -> p s e", p=128)

    with ExitStack() as persist:
        ppool = persist.enter_context(tc.tile_pool(name="persist", bufs=1))
        # PE warmup: junk matmuls run during the input-DMA wait to open the
        # HAM clock gate
        warm_sb = ppool.tile([128, 640], BF16)
        nc.vector.memset(warm_sb[:], 1.0)
        wp_sb = ppool.tile([128, MC, EMB], BF16)
        qT_sb = ppool.tile([128, MC, N], BF16)
        kT_sb = ppool.tile([128, MC, N], BF16)
        # per head block: [v columns (64) | ones columns (64)] so the AV
        # matmul also produces l replicated across 64 psum rows
        v_sb = ppool.tile([128, NK, HL * 2 * D], BF16)
        nc.vector.memset(
            v_sb[:].rearrange("p k (h c) -> p k h c", c=2 * D)[:, :, :, D:2 * D],
            1.0)
        attT_sb = ppool.tile([128, MC, N], BF16)

        psum_pool = persist.enter_context(
            tc.tile_pool(name="psum", bufs=1, space="PSUM"))
        warm_ps = psum_pool.tile([128, 512], F32, tag="av", bufs=2, name="warm_ps")
        for wi in range(10):
            nc.tensor.matmul(warm_ps[:], warm_sb[:, 0:128], warm_sb[:, 128:640],
                             start=(wi == 0), stop=(wi == 9))

        # ---- phases 1+2+3 share a scope (x/w stay resident through
        # phase 1; o_sb staging lives through phase 2's interleaved
        # output projection) ----
        with ExitStack() as ph2:
            p1 = ph2.enter_context(tc.tile_pool(name="ph1", bufs=1))
            x_sb = p1.tile([128, EC, N], BF16)
            w_sb = p1.tile([128, EC, 3 * HD], BF16)
            for c in range(EC):
                nc.sync.dma_start(w_sb[:, c, :], wTr[:, c, :])
                nc.sync.dma_start(x_sb[:, c, :], xTr[:, c, :])
            # wp is only needed in phase 3; don't put it ahead of x/w
            nc.sync.dma_start(wp_sb[:], wpTr)

            # phase 1, c-outermost over groups of 3 q/k blocks (the eps
            # psum ring depth): every input chunk c is consumed by all
            # in-flight accumulators as soon as its DMA lands, so compute
            # tracks the ~13us input stream instead of serializing on the
            # last chunk. Two v accumulators ride along per group; the
            # remaining v tiles run after the stream (inputs resident).
            v_state = {}

            def v_step(s, c):
                if s not in v_state:
                    v_state[s] = psum_pool.tile([128, 512], F32, tag="av",
                                                bufs=2, name=f"vv_{s}")[:, 0:HD]
                nc.tensor.matmul(
                    v_state[s],
                    (x_sb[:, c, s * 128:(s + 1) * 128]),
                    (w_sb[:, c, 2 * HD:3 * HD]),
                    start=(c == 0), stop=(c == EC - 1))
                if c == EC - 1:
                    nc.vector.tensor_copy(
                        v_sb[:, s, :].rearrange(
                            "p (h c) -> p h c", c=2 * D)[:, :, 0:D],
                        v_state.pop(s)[:].rearrange("p (h d) -> p h d", h=HL))

            blocks = [(which, m, nh) for which in (0, 1)
                      for m in range(MC) for nh in (0, 1)]
            for g in range(6):
                group = blocks[2 * g:2 * g + 2]
                mm4s = {}
                for b, (which, m, nh) in enumerate(group):
                    mm4s[b] = psum_pool.tile([128, 2, 512], F32, tag="eps",
                                             bufs=3, name=f"mm4_{g}_{b}")
                for c in range(EC):
                    for b, (which, m, nh) in enumerate(group):
                        lo = which * HD + m * 128
                        for j in (0, 1):
                            n = 2 * nh + j
                            nc.tensor.matmul(
                                mm4s[b][:, j, :],
                                (w_sb[:, c, lo:lo + 128]),
                                (x_sb[:, c, n * 512:(n + 1) * 512]),
                                start=(c == 0), stop=(c == EC - 1))
                    v_step(2 * g, c)
                    v_step(2 * g + 1, c)
                for b, (which, m, nh) in enumerate(group):
                    dst = qT_sb if which == 0 else kT_sb
                    for j in (0, 1):
                        n = 2 * nh + j
                        ns = slice(n * 512, (n + 1) * 512)
                        if (which + n) % 2 == 0:
                            nc.scalar.copy(dst[:, m, ns], mm4s[b][:, j, :])
                        else:
                            nc.vector.tensor_copy(dst[:, m, ns], mm4s[b][:, j, :])
            for s0 in range(12, NK, 2):
                for c in range(EC):
                    v_step(s0, c)
                    v_step(s0 + 1, c)

            esb_pool = ph2.enter_context(tc.tile_pool(name="esb", bufs=4))
            sm_pool = ph2.enter_context(tc.tile_pool(name="sm", bufs=4))
            osb_pool = ph2.enter_context(tc.tile_pool(name="osb", bufs=3))

            def make_pr_jobs(n):
                # output projection for query group n: 4 s-chunks x 2
                # halves sharing the eps psum ring; DMA fires per s-chunk
                jobs = []
                for s in range(4 * n, 4 * n + 4):
                    o_sb = osb_pool.tile([128, EMB], BF16, tag="osb",
                                         name=f"osb_{s}")
                    for half in range(2):
                        def job(s=s, half=half, o_sb=o_sb):
                            pr = psum_pool.tile([128, 2, 512], F32, tag="eps",
                                                bufs=3,
                                                name=f"pr_{s}_{half}")[:, 0, 0:HD]
                            for mm in range(MC):
                                nc.tensor.matmul(
                                    pr[:],
                                    (attT_sb[:, mm, s * 128:(s + 1) * 128]),
                                    (wp_sb[:, mm, half * HD:(half + 1) * HD]),
                                    start=(mm == 0), stop=(mm == MC - 1))
                            if half == 0:
                                nc.vector.tensor_copy(o_sb[:, 0:HD], pr[:])
                            else:
                                nc.scalar.copy(o_sb[:, HD:2 * HD], pr[:])
                                nc.sync.dma_start(outr[:, s, :], o_sb[:])
                        jobs.append(job)
                return jobs

            pr_queue = []
            pending_norm = {}
            for n in range(NQ):
                for m in range(MC):
                    unit = n * MC + m
                    ns = slice(n * 512, (n + 1) * 512)
                    kslice = lambda kk: slice(kk * 128, (kk + 1) * 128)
                    av_t = [psum_pool.tile([128, 512], F32, tag="av", bufs=2,
                                           name=f"av_{m}_{n}_{z}")
                            for z in (0, 1)]
                    mvq = []

                    def emit_av(j):
                        for z in (0, 1):
                            h = 2 * m + z
                            nc.tensor.matmul(
                                av_t[z][:],
                                (v_sb[:, j, h * 2 * D:(h + 1) * 2 * D]),
                                mvq[j][z],
                                start=(j == 0), stop=(j == NK - 1))

                    for kk in range(NK):
                        if kk >= DEPTH:
                            emit_av(kk - DEPTH)
                        e2 = psum_pool.tile([128, 2, 512], F32, tag="eps",
                                            bufs=3, name=f"e_{m}_{n}_{kk}")
                        nc.tensor.matmul(e2[:, 0, :],
                                         (kT_sb[0:64, m, kslice(kk)]),
                                         (qT_sb[0:64, m, ns]),
                                         start=True, stop=True)
                        nc.tensor.matmul(e2[:, 1, :],
                                         (kT_sb[64:128, m, kslice(kk)]),
                                         (qT_sb[64:128, m, ns]),
                                         start=True, stop=True)
                        # exp FIRST (ring-critical), then the deferred
                        # norm ops: their deps are a unit old, but emitting
                        # them earlier would queue them ahead of the exp on
                        # the same engine FIFO and delay the e2 ring.
                        if kk % 2 == 1:
                            esb = esb_pool.tile([128, 2, 512], BF16, tag="esb",
                                                bufs=7, name=f"esb_{m}_{n}_{kk}")
                            nc.scalar.activation(esb[:], e2[:], EXP, scale=SCALE)
                            mvq.append((esb[:, 0, :], esb[:, 1, :]))
                        else:
                            esf = esb_pool.tile([128, 2, 512], F32, tag="esf",
                                                bufs=7, name=f"esf_{m}_{n}_{kk}")
                            nc.vector.tensor_scalar(esf[:], e2[:], ASC, BMAGIC,
                                                    MULT, ADD)
                            bv = esf[:].bitcast(I16)[:, :, 0::2].bitcast(BF16)
                            mvq.append((bv[:, 0, :], bv[:, 1, :]))
                        if kk in pending_norm:
                            for fn in pending_norm.pop(kk):
                                fn()
                    for j in range(NK - DEPTH, NK):
                        emit_av(j)
                        if j in (11, 13, 15) and pr_queue:
                            pr_queue.pop(0)()

                    # drain + normalize for this (pair, n):
                    # rows 0:64 = av, rows 64:128 = l replicated 64-wide.
                    # copies split 3 ACT / 1 DVE; 1/l on DVE; the attT
                    # multiplies run on GPSIMD (own queue, off the exp path).
                    # All ops are spread into the next unit's first kk slots
                    # so neither engine sees a boundary burst.
                    tiles = []
                    for z in (0, 1):
                        tiles.append((
                            sm_pool.tile([D, 512], F32, tag=f"avst{z}",
                                         bufs=3, name=f"avst_{m}_{n}_{z}"),
                            sm_pool.tile([D, 512], F32, tag=f"lrep{z}",
                                         bufs=3, name=f"lrep_{m}_{n}_{z}"),
                            sm_pool.tile([D, 512], F32, tag=f"rb{z}",
                                         bufs=3, name=f"rb_{m}_{n}_{z}")))

                    def norm_ops(m=m, ns=ns, av_t=av_t, tiles=tiles):
                        a0, l0, r0 = tiles[0]
                        a1, l1, r1 = tiles[1]

                        def mul(z, a, r):
                            nc.gpsimd.tensor_mul(
                                attT_sb[z * 64:(z + 1) * 64, m, ns], a[:], r[:])
                        return {
                            1: [lambda: nc.scalar.copy(a0[:], av_t[0][0:D, :])],
                            3: [lambda: nc.scalar.copy(a1[:], av_t[1][0:D, :])],
                            5: [lambda: nc.vector.tensor_copy(
                                    l0[:], av_t[0][D:2 * D, :])],
                            7: [lambda: nc.scalar.copy(l1[:],
                                                       av_t[1][D:2 * D, :]),
                                lambda: nc.vector.reciprocal_approx_fast(
                                    r0[:], l0[:])],
                            9: [lambda: nc.vector.reciprocal_approx_fast(
                                    r1[:], l1[:]),
                                lambda: mul(0, a0, r0)],
                            11: [lambda: mul(1, a1, r1)],
                        }

                    if unit == MC * NQ - 1:
                        for kk, fns in sorted(norm_ops().items()):
                            for fn in fns:
                                fn()
                    else:
                        pending_norm = norm_ops()
                if n > 0:
                    pr_queue.extend(make_pr_jobs(n - 1))

            # keep the PE busy through the last unit's normalization, then
            # run the last group's output projection
            fill_ps = psum_pool.tile([128, 512], F32, tag="av", bufs=2,
                                     name="fill_ps")
            for wi in range(16):
                nc.tensor.matmul(fill_ps[:], warm_sb[:, 0:128],
                                 warm_sb[:, 128:640],
                                 start=(wi == 0), stop=(wi == 15))
            pr_queue.extend(make_pr_jobs(NQ - 1))
            while pr_queue:
                pr_queue.pop(0)()


_CACHE = {}


def _build():
    if "nc" not in _CACHE:
        nc = bacc.Bacc("TRN2", target_bir_lowering=False, debug=False,
                       num_devices=NCORES)
        with tile.TileContext(nc) as tc:
            _emit(tc)
        nc.compile()
        _CACHE["nc"] = nc
    return _CACHE["nc"]


def _in_maps(x, W_qkv, W_proj):
    in_maps = []
    for c in range(NCORES):
        b, g = divmod(c, 2)
        r0 = g * HD
        w_rows = np.concatenate([
            W_qkv[0 * EMB + r0: 0 * EMB + r0 + HD],
            W_qkv[1 * EMB + r0: 1 * EMB + r0 + HD],
            W_qkv[2 * EMB + r0: 2 * EMB + r0 + HD],
        ], axis=0)                                   # [1152, 768]
        bf = ml_dtypes.bfloat16
        in_maps.append({
            "xT": np.ascontiguousarray(x[b].T.astype(bf)),
            "wT": np.ascontiguousarray(w_rows.T.astype(bf)),
            "wpT": np.ascontiguousarray(W_proj[:, r0:r0 + HD].T.astype(bf)),
        })
    return in_maps


LAST_RESULTS = None


def kernel(x, W_qkv, W_proj, b_proj):
    global LAST_RESULTS
    x = np.ascontiguousarray(np.asarray(x, dtype=np.float32))
    W_qkv = np.asarray(W_qkv, dtype=np.float32)
    W_proj = np.asarray(W_proj, dtype=np.float32)
    b_proj = np.asarray(b_proj, dtype=np.float32)

    nc = _build()
    in_maps = _in_maps(x, W_qkv, W_proj)
    res = run_bass_kernel_spmd(nc, in_maps, core_ids=list(range(NCORES)))
    LAST_RESULTS = res

    out = np.empty((B, N, EMB), dtype=np.float32)
    for b in range(B):
        out[b] = (res.results[2 * b]["outp"].astype(np.float32)
                  + res.results[2 * b + 1]["outp"].astype(np.float32))
    out += b_proj
    return out



# revision 37
# speedup vs baseline: 1.2245x; 1.0125x over previous
"""MultiHeadAttention Trainium2 kernel.

Full inputs: x [4, 2048, 768] f32, W_qkv [2304, 768], W_proj [768, 768],
b_proj [768]. Output [4, 2048, 768] f32.

Sharding: 8 cores = 4 batches x 2 head-groups (6 heads each).
Per-core inputs (host-prepared, transposed on host):
  xT  [768, 2048]  = x[b].T
  wT  [768, 1152]  = concat(Wq_g, Wk_g, Wv_g).T   (g = head group rows)
  wpT [384, 768]   = W_proj[:, g-cols].T
Per-core output: outp [2048, 768] = partial projection output for batch b.
Host: out[b] = outp[2b] + outp[2b+1] + b_proj.

Key optimizations vs the 332.9us baseline (275us -> ~258us this session):
  - QK matmuls run as row-tiled pairs (tile_position inferred from base
    partitions 0:64 / 64:128): two K=64 matmuls execute concurrently in
    the PE array, halving QK cost. kT_sb stores head pairs like qT_sb.
  - The softmax exp is split between the Scalar engine (real Exp LUT,
    odd kk chunks) and the Vector engine (even chunks; Schraudolph
    bit-trick: y = E*(128/ln2/8) + bias + 2^23 in f32 — the low 16 bits
    of y's mantissa ARE the bf16 of e^E; the AV matmul reads them as a
    stride-2 bf16 view). Both engines are PSUM-read-port bound (ACT
    (172+1024)/1.2 = 997ns/chunk, DVE (120+1024)/0.96 = 1192ns), so
    phase 2 is exp-engine bound at ~12us/unit. NOTE: the strict 8:8
    odd/even alternation is load-bearing — any cadence change (9:7
    split, per-chunk column offload, split-engine boundary chunks,
    cross-engine same-slot deps) collapses the pipeline by 10-50us.
  - The AV stationary carries the 64 v columns PLUS 64 ones columns, so
    av psum rows 64:128 hold the softmax denominator l replicated
    64-wide; reciprocal_approx_fast needs a base-partition-0 SBUF input
    (custom-DVE ops misbehave at base partition 64), so l is copied
    down first. attT multiplies run on GPSIMD (no PSUM port, so the av
    rows must be copied to SBUF for it).
  - Norm work is spread one op per 2 kk slots (kks 1..11) of the next
    unit so neither exp engine ever sees a burst that delays the
    ring-critical exps.
  - Phase 2 emits the AV pair BEFORE the QK pair in each kk slot: the
    AVs' deps are old, so they stream while the QK pair waits on the
    3-deep e2 psum ring (PSUM: 2 av accumulators + 3x2 energy tiles).
  - Units run n-outer; the output projection for query group g runs as
    8 half-blocks inserted into the AV-drain regions of group g+1's
    units (sharing the eps psum ring), so the output DMA overlaps
    phase-2 compute. Output partials are written as bf16 (halves the
    6.3MB->3.15MB output DMA; the host upcasts and sums).
  - Phase 1 runs c-outermost over groups of 2 q/k blocks (eps ring is
    3-deep, so group g+1 never waits on group g's drains), consuming
    each 820KB input chunk as its DMA lands instead of serializing on
    the last chunk; 2 v accumulators (in the phase-1-idle av psum tag)
    ride along per group, the rest run after the stream.
  - wp (phase-3 weights) DMA is deferred past the x/w input DMAs.
"""

import ml_dtypes
import numpy as np

import concourse.bass as bass
import concourse.tile as tile
from concourse import bacc, mybir
from concourse.bass_utils import run_bass_kernel_spmd

EMB = 768
N = 2048
B = 4
D = 64
HL = 6            # heads per core
HD = HL * D       # 384 local head-dim columns
NCORES = 8
SCALE = D ** -0.5

F32 = mybir.dt.float32
BF16 = mybir.dt.bfloat16
I16 = mybir.dt.int16

EC = EMB // 128   # 6 emb chunks
MC = HD // 128    # 3 head pairs
NQ = N // 512     # 4 query chunks of 512
NK = N // 128     # 16 key chunks of 128
DEPTH = 8         # AV software-pipeline depth (in kk steps)

EXP = mybir.ActivationFunctionType.Exp
MULT = mybir.AluOpType.mult
ADD = mybir.AluOpType.add

ASC = float(128.0 / np.log(2) * SCALE)      # schraudolph slope (scale folded)
BMAGIC = float(16250.5 + 2 ** 23)           # schraudolph bias + f32 round trick


def _emit(tc):
    from contextlib import ExitStack

    nc = tc.nc
    xT = nc.dram_tensor("xT", [EMB, N], BF16, kind="ExternalInput").ap()
    wT = nc.dram_tensor("wT", [EMB, 3 * HD], BF16, kind="ExternalInput").ap()
    wpT = nc.dram_tensor("wpT", [HD, EMB], BF16, kind="ExternalInput").ap()
    outp = nc.dram_tensor("outp", [N, EMB], BF16, kind="ExternalOutput").ap()

    xTr = xT.rearrange("(c p) s -> p c s", p=128)
    wTr = wT.rearrange("(c p) s -> p c s", p=128)
    wpTr = wpT.rearrange("(m p) e -> p m e", p=128)
    outr = outp.rearrange("(s p) e -> p s e", p=128)

    with ExitStack() as persist:
        ppool = persist.enter_context(tc.tile_pool(name="persist", bufs=1))
        # PE warmup: junk matmuls run during the input-DMA wait to open the
        # HAM clock gate
        warm_sb = ppool.tile([128, 640], BF16)
        nc.vector.memset(warm_sb[:], 1.0)
        wp_sb = ppool.tile([128, MC, EMB], BF16)
        qT_sb = ppool.tile([128, MC, N], BF16)
        kT_sb = ppool.tile([128, MC, N], BF16)
        # per head block: [v columns (64) | ones columns (64)] so the AV
        # matmul also produces l replicated across 64 psum rows
        v_sb = ppool.tile([128, NK, HL * 2 * D], BF16)
        nc.vector.memset(
            v_sb[:].rearrange("p k (h c) -> p k h c", c=2 * D)[:, :, :, D:2 * D],
            1.0)
        attT_sb = ppool.tile([128, MC, N], BF16)

        psum_pool = persist.enter_context(
            tc.tile_pool(name="psum", bufs=1, space="PSUM"))
        warm_ps = psum_pool.tile([128, 512], F32, tag="av", bufs=2, name="warm_ps")
        for wi in range(10):
            nc.tensor.matmul(warm_ps[:], warm_sb[:, 0:128], warm_sb[:, 128:640],
                             start=(wi == 0), stop=(wi == 9))

        # ---- phases 1+2+3 share a scope (x/w stay resident through
        # phase 1; o_sb staging lives through phase 2's interleaved
        # output projection) ----
        with ExitStack() as ph2:
            p1 = ph2.enter_context(tc.tile_pool(name="ph1", bufs=1))
            x_sb = p1.tile([128, EC, N], BF16)
            w_sb = p1.tile([128, EC, 3 * HD], BF16)
            for c in range(EC):
                nc.sync.dma_start(w_sb[:, c, :], wTr[:, c, :])
                nc.sync.dma_start(x_sb[:, c, :], xTr[:, c, :])
            # wp is only needed in phase 3; don't put it ahead of x/w
            nc.sync.dma_start(wp_sb[:], wpTr)

            # phase 1, c-outermost over groups of 3 q/k blocks (the eps
            # psum ring depth): every input chunk c is consumed by all
            # in-flight accumulators as soon as its DMA lands, so compute
            # tracks the ~13us input stream instead of serializing on the
            # last chunk. Two v accumulators ride along per group; the
            # remaining v tiles run after the stream (inputs resident).
            v_state = {}

            def v_step(s, c):
                if s not in v_state:
                    v_state[s] = psum_pool.tile([128, 512], F32, tag="av",
                                                bufs=2, name=f"vv_{s}")[:, 0:HD]
                nc.tensor.matmul(
                    v_state[s],
                    (x_sb[:, c, s * 128:(s + 1) * 128]),
                    (w_sb[:, c, 2 * HD:3 * HD]),
                    start=(c == 0), stop=(c == EC - 1))
                if c == EC - 1:
                    nc.vector.tensor_copy(
                        v_sb[:, s, :].rearrange(
                            "p (h c) -> p h c", c=2 * D)[:, :, 0:D],
                        v_state.pop(s)[:].rearrange("p (h d) -> p h d", h=HL))

            blocks = [(which, m, nh) for which in (0, 1)
                      for m in range(MC) for nh in (0, 1)]
            for g in range(6):
                group = blocks[2 * g:2 * g + 2]
                mm4s = {}
                for b, (which, m, nh) in enumerate(group):
                    mm4s[b] = psum_pool.tile([128, 2, 512], F32, tag="eps",
                                             bufs=3, name=f"mm4_{g}_{b}")
                for c in range(EC):
                    for b, (which, m, nh) in enumerate(group):
                        lo = which * HD + m * 128
                        for j in (0, 1):
                            n = 2 * nh + j
                            nc.tensor.matmul(
                                mm4s[b][:, j, :],
                                (w_sb[:, c, lo:lo + 128]),
                                (x_sb[:, c, n * 512:(n + 1) * 512]),
                                start=(c == 0), stop=(c == EC - 1))
                    v_step(2 * g, c)
                    v_step(2 * g + 1, c)
                for b, (which, m, nh) in enumerate(group):
                    dst = qT_sb if which == 0 else kT_sb
                    for j in (0, 1):
                        n = 2 * nh + j
                        ns = slice(n * 512, (n + 1) * 512)
                        if (which + n) % 2 == 0:
                            nc.scalar.copy(dst[:, m, ns], mm4s[b][:, j, :])
                        else:
                            nc.vector.tensor_copy(dst[:, m, ns], mm4s[b][:, j, :])
            for s0 in range(12, NK, 2):
                for c in range(EC):
                    v_step(s0, c)
                    v_step(s0 + 1, c)

            esb_pool = ph2.enter_context(tc.tile_pool(name="esb", bufs=4))
            sm_pool = ph2.enter_context(tc.tile_pool(name="sm", bufs=4))
            osb_pool = ph2.enter_context(tc.tile_pool(name="osb", bufs=3))

            def make_pr_jobs(n):
                # output projection for query group n: 4 s-chunks x 2
                # halves sharing the eps psum ring; DMA fires per s-chunk
                jobs = []
                for s in range(4 * n, 4 * n + 4):
                    o_sb = osb_pool.tile([128, EMB], BF16, tag="osb",
                                         name=f"osb_{s}")
                    for half in range(2):
                        def job(s=s, half=half, o_sb=o_sb):
                            pr = psum_pool.tile([128, 2, 512], F32, tag="eps",
                                                bufs=3,
                                                name=f"pr_{s}_{half}")[:, 0, 0:HD]
                            for mm in range(MC):
                                nc.tensor.matmul(
                                    pr[:],
                                    (attT_sb[:, mm, s * 128:(s + 1) * 128]),
                                    (wp_sb[:, mm, half * HD:(half + 1) * HD]),
                                    start=(mm == 0), stop=(mm == MC - 1))
                            if half == 0:
                                nc.vector.tensor_copy(o_sb[:, 0:HD], pr[:])
                            else:
                                nc.scalar.copy(o_sb[:, HD:2 * HD], pr[:])
                                nc.sync.dma_start(outr[:, s, :], o_sb[:])
                        jobs.append(job)
                return jobs

            pr_queue = []
            pending_norm = {}
            for n in range(NQ):
                for m in range(MC):
                    unit = n * MC + m
                    ns = slice(n * 512, (n + 1) * 512)
                    kslice = lambda kk: slice(kk * 128, (kk + 1) * 128)
                    av_t = [psum_pool.tile([128, 512], F32, tag="av", bufs=2,
                                           name=f"av_{m}_{n}_{z}")
                            for z in (0, 1)]
                    mvq = []

                    def emit_av(j):
                        for z in (0, 1):
                            h = 2 * m + z
                            nc.tensor.matmul(
                                av_t[z][:],
                                (v_sb[:, j, h * 2 * D:(h + 1) * 2 * D]),
                                mvq[j][z],
                                start=(j == 0), stop=(j == NK - 1))

                    for kk in range(NK):
                        if kk >= DEPTH:
                            emit_av(kk - DEPTH)
                        e2 = psum_pool.tile([128, 2, 512], F32, tag="eps",
                                            bufs=3, name=f"e_{m}_{n}_{kk}")
                        nc.tensor.matmul(e2[:, 0, :],
                                         (kT_sb[0:64, m, kslice(kk)]),
                                         (qT_sb[0:64, m, ns]),
                                         start=True, stop=True)
                        nc.tensor.matmul(e2[:, 1, :],
                                         (kT_sb[64:128, m, kslice(kk)]),
                                         (qT_sb[64:128, m, ns]),
                                         start=True, stop=True)
                        # exp FIRST (ring-critical), then the deferred
                        # norm ops: their deps are a unit old, but emitting
                        # them earlier would queue them ahead of the exp on
                        # the same engine FIFO and delay the e2 ring.
                        if kk % 2 == 1:
                            esb = esb_pool.tile([128, 2, 512], BF16, tag="esb",
                                                bufs=7, name=f"esb_{m}_{n}_{kk}")
                            nc.scalar.activation(esb[:], e2[:], EXP, scale=SCALE)
                            mvq.append((esb[:, 0, :], esb[:, 1, :]))
                        else:
                            esf = esb_pool.tile([128, 2, 512], F32, tag="esf",
                                                bufs=7, name=f"esf_{m}_{n}_{kk}")
                            nc.vector.tensor_scalar(esf[:], e2[:], ASC, BMAGIC,
                                                    MULT, ADD)
                            bv = esf[:].bitcast(I16)[:, :, 0::2].bitcast(BF16)
                            mvq.append((bv[:, 0, :], bv[:, 1, :]))
                        if kk in pending_norm:
                            for fn in pending_norm.pop(kk):
                                fn()
                    for j in range(NK - DEPTH, NK):
                        emit_av(j)
                        if j in (11, 13, 15) and pr_queue:
                            pr_queue.pop(0)()

                    # drain + normalize for this (pair, n):
                    # rows 0:64 = av, rows 64:128 = l replicated 64-wide.
                    # copies split 3 ACT / 1 DVE; 1/l on DVE; the attT
                    # multiplies run on GPSIMD (own queue, off the exp path).
                    # All ops are spread into the next unit's first kk slots
                    # so neither engine sees a boundary burst.
                    tiles = []
                    for z in (0, 1):
                        tiles.append((
                            sm_pool.tile([D, 512], F32, tag=f"avst{z}",
                                         bufs=3, name=f"avst_{m}_{n}_{z}"),
                            sm_pool.tile([D, 512], F32, tag=f"lrep{z}",
                                         bufs=3, name=f"lrep_{m}_{n}_{z}"),
                            sm_pool.tile([D, 512], F32, tag=f"rb{z}",
                                         bufs=3, name=f"rb_{m}_{n}_{z}")))

                    def norm_ops(m=m, ns=ns, av_t=av_t, tiles=tiles):
                        a0, l0, r0 = tiles[0]
                        a1, l1, r1 = tiles[1]

                        def mul(z, a, r):
                            nc.gpsimd.tensor_mul(
                                attT_sb[z * 64:(z + 1) * 64, m, ns], a[:], r[:])
                        return {
                            1: [lambda: nc.scalar.copy(a0[:], av_t[0][0:D, :])],
                            3: [lambda: nc.scalar.copy(a1[:], av_t[1][0:D, :])],
                            5: [lambda: nc.vector.tensor_copy(
                                    l0[:], av_t[0][D:2 * D, :])],
                            7: [lambda: nc.scalar.copy(l1[:],
                                                       av_t[1][D:2 * D, :]),
                                lambda: nc.vector.reciprocal_approx_fast(
                                    r0[:], l0[:])],
                            9: [lambda: nc.vector.reciprocal_approx_fast(
                                    r1[:], l1[:]),
                                lambda: mul(0, a0, r0)],
                            11: [lambda: mul(1, a1, r1)],
                        }

                    if unit == MC * NQ - 1:
                        for kk, fns in sorted(norm_ops().items()):
                            for fn in fns:
                                fn()
                    else:
                        pending_norm = norm_ops()
                if n > 0:
                    pr_queue.extend(make_pr_jobs(n - 1))

            # keep the PE busy through the last unit's normalization, then
            # run the last group's output projection
            fill_ps = psum_pool.tile([128, 512], F32, tag="av", bufs=2,
                                     name="fill_ps")
            for wi in range(4):
                nc.tensor.matmul(fill_ps[:], warm_sb[:, 0:128],
                                 warm_sb[:, 128:640],
                                 start=(wi == 0), stop=(wi == 3))
            pr_queue.extend(make_pr_jobs(NQ - 1))
            while pr_queue:
                pr_queue.pop(0)()


_CACHE = {}


def _build():
    if "nc" not in _CACHE:
        nc = bacc.Bacc("TRN2", target_bir_lowering=False, debug=False,
                       num_devices=NCORES)
        with tile.TileContext(nc) as tc:
            _emit(tc)
        nc.compile()
        _CACHE["nc"] = nc
    return _CACHE["nc"]


def _in_maps(x, W_qkv, W_proj):
    in_maps = []
    for c in range(NCORES):
        b, g = divmod(c, 2)
        r0 = g * HD
        w_rows = np.concatenate([
            W_qkv[0 * EMB + r0: 0 * EMB + r0 + HD],
            W_qkv[1 * EMB + r0: 1 * EMB + r0 + HD],
            W_qkv[2 * EMB + r0: 2 * EMB + r0 + HD],
        ], axis=0)                                   # [1152, 768]
        bf = ml_dtypes.bfloat16
        in_maps.append({
            "xT": np.ascontiguousarray(x[b].T.astype(bf)),
            "wT": np.ascontiguousarray(w_rows.T.astype(bf)),
            "wpT": np.ascontiguousarray(W_proj[:, r0:r0 + HD].T.astype(bf)),
        })
    return in_maps


LAST_RESULTS = None


def kernel(x, W_qkv, W_proj, b_proj):
    global LAST_RESULTS
    x = np.ascontiguousarray(np.asarray(x, dtype=np.float32))
    W_qkv = np.asarray(W_qkv, dtype=np.float32)
    W_proj = np.asarray(W_proj, dtype=np.float32)
    b_proj = np.asarray(b_proj, dtype=np.float32)

    nc = _build()
    in_maps = _in_maps(x, W_qkv, W_proj)
    res = run_bass_kernel_spmd(nc, in_maps, core_ids=list(range(NCORES)))
    LAST_RESULTS = res

    out = np.empty((B, N, EMB), dtype=np.float32)
    for b in range(B):
        out[b] = (res.results[2 * b]["outp"].astype(np.float32)
                  + res.results[2 * b + 1]["outp"].astype(np.float32))
    out += b_proj
    return out



# revision 38
# speedup vs baseline: 1.2289x; 1.0035x over previous
"""MultiHeadAttention Trainium2 kernel.

Full inputs: x [4, 2048, 768] f32, W_qkv [2304, 768], W_proj [768, 768],
b_proj [768]. Output [4, 2048, 768] f32.

Sharding: 8 cores = 4 batches x 2 head-groups (6 heads each).
Per-core inputs (host-prepared, transposed on host):
  xT  [768, 2048]  = x[b].T
  wT  [768, 1152]  = concat(Wq_g, Wk_g, Wv_g).T   (g = head group rows)
  wpT [384, 768]   = W_proj[:, g-cols].T
Per-core output: outp [2048, 768] = partial projection output for batch b.
Host: out[b] = outp[2b] + outp[2b+1] + b_proj.

Key optimizations vs the 332.9us baseline (275us -> ~258us this session):
  - QK matmuls run as row-tiled pairs (tile_position inferred from base
    partitions 0:64 / 64:128): two K=64 matmuls execute concurrently in
    the PE array, halving QK cost. kT_sb stores head pairs like qT_sb.
  - The softmax exp is split between the Scalar engine (real Exp LUT,
    odd kk chunks) and the Vector engine (even chunks; Schraudolph
    bit-trick: y = E*(128/ln2/8) + bias + 2^23 in f32 — the low 16 bits
    of y's mantissa ARE the bf16 of e^E; the AV matmul reads them as a
    stride-2 bf16 view). Both engines are PSUM-read-port bound (ACT
    (172+1024)/1.2 = 997ns/chunk, DVE (120+1024)/0.96 = 1192ns), so
    phase 2 is exp-engine bound at ~12us/unit. NOTE: the strict 8:8
    odd/even alternation is load-bearing — any cadence change (9:7
    split, per-chunk column offload, split-engine boundary chunks,
    cross-engine same-slot deps) collapses the pipeline by 10-50us.
  - The AV stationary carries the 64 v columns PLUS 64 ones columns, so
    av psum rows 64:128 hold the softmax denominator l replicated
    64-wide; reciprocal_approx_fast needs a base-partition-0 SBUF input
    (custom-DVE ops misbehave at base partition 64), so l is copied
    down first. attT multiplies run on GPSIMD (no PSUM port, so the av
    rows must be copied to SBUF for it).
  - Norm work is spread one op per 2 kk slots (kks 1..11) of the next
    unit so neither exp engine ever sees a burst that delays the
    ring-critical exps.
  - Phase 2 emits the AV pair BEFORE the QK pair in each kk slot: the
    AVs' deps are old, so they stream while the QK pair waits on the
    3-deep e2 psum ring (PSUM: 2 av accumulators + 3x2 energy tiles).
  - Units run n-outer; the output projection for query group g runs as
    8 half-blocks inserted into the AV-drain regions of group g+1's
    units (sharing the eps psum ring), so the output DMA overlaps
    phase-2 compute. Output partials are written as bf16 (halves the
    6.3MB->3.15MB output DMA; the host upcasts and sums).
  - Phase 1 runs c-outermost over groups of 2 q/k blocks (eps ring is
    3-deep, so group g+1 never waits on group g's drains), consuming
    each 820KB input chunk as its DMA lands instead of serializing on
    the last chunk; 2 v accumulators (in the phase-1-idle av psum tag)
    ride along per group, the rest run after the stream.
  - wp (phase-3 weights) DMA is deferred past the x/w input DMAs.
"""

import ml_dtypes
import numpy as np

import concourse.bass as bass
import concourse.tile as tile
from concourse import bacc, mybir
from concourse.bass_utils import run_bass_kernel_spmd

EMB = 768
N = 2048
B = 4
D = 64
HL = 6            # heads per core
HD = HL * D       # 384 local head-dim columns
NCORES = 8
SCALE = D ** -0.5

F32 = mybir.dt.float32
BF16 = mybir.dt.bfloat16
I16 = mybir.dt.int16

EC = EMB // 128   # 6 emb chunks
MC = HD // 128    # 3 head pairs
NQ = N // 512     # 4 query chunks of 512
NK = N // 128     # 16 key chunks of 128
DEPTH = 8         # AV software-pipeline depth (in kk steps)

EXP = mybir.ActivationFunctionType.Exp
MULT = mybir.AluOpType.mult
ADD = mybir.AluOpType.add

ASC = float(128.0 / np.log(2) * SCALE)      # schraudolph slope (scale folded)
BMAGIC = float(16250.5 + 2 ** 23)           # schraudolph bias + f32 round trick


def _emit(tc):
    from contextlib import ExitStack

    nc = tc.nc
    xT = nc.dram_tensor("xT", [EMB, N], BF16, kind="ExternalInput").ap()
    wT = nc.dram_tensor("wT", [EMB, 3 * HD], BF16, kind="ExternalInput").ap()
    wpT = nc.dram_tensor("wpT", [HD, EMB], BF16, kind="ExternalInput").ap()
    outp = nc.dram_tensor("outp", [N, EMB], BF16, kind="ExternalOutput").ap()

    xTr = xT.rearrange("(c p) s -> p c s", p=128)
    wTr = wT.rearrange("(c p) s -> p c s", p=128)
    wpTr = wpT.rearrange("(m p) e -> p m e", p=128)
    outr = outp.rearrange("(s p) e -> p s e", p=128)

    with ExitStack() as persist:
        ppool = persist.enter_context(tc.tile_pool(name="persist", bufs=1))
        # PE warmup: junk matmuls run during the input-DMA wait to open the
        # HAM clock gate
        warm_sb = ppool.tile([128, 640], BF16)
        nc.vector.memset(warm_sb[:], 1.0)
        wp_sb = ppool.tile([128, MC, EMB], BF16)
        qT_sb = ppool.tile([128, MC, N], BF16)
        kT_sb = ppool.tile([128, MC, N], BF16)
        # per head block: [v columns (64) | ones columns (64)] so the AV
        # matmul also produces l replicated across 64 psum rows
        v_sb = ppool.tile([128, NK, HL * 2 * D], BF16)
        nc.vector.memset(
            v_sb[:].rearrange("p k (h c) -> p k h c", c=2 * D)[:, :, :, D:2 * D],
            1.0)
        attT_sb = ppool.tile([128, MC, N], BF16)

        psum_pool = persist.enter_context(
            tc.tile_pool(name="psum", bufs=1, space="PSUM"))
        warm_ps = psum_pool.tile([128, 512], F32, tag="av", bufs=2, name="warm_ps")
        for wi in range(10):
            nc.tensor.matmul(warm_ps[:], warm_sb[:, 0:128], warm_sb[:, 128:640],
                             start=(wi == 0), stop=(wi == 9))

        # ---- phases 1+2+3 share a scope (x/w stay resident through
        # phase 1; o_sb staging lives through phase 2's interleaved
        # output projection) ----
        with ExitStack() as ph2:
            p1 = ph2.enter_context(tc.tile_pool(name="ph1", bufs=1))
            x_sb = p1.tile([128, EC, N], BF16)
            w_sb = p1.tile([128, EC, 3 * HD], BF16)
            for c in range(EC):
                nc.sync.dma_start(w_sb[:, c, :], wTr[:, c, :])
                nc.sync.dma_start(x_sb[:, c, :], xTr[:, c, :])
            # wp is only needed in phase 3; don't put it ahead of x/w
            nc.sync.dma_start(wp_sb[:], wpTr)

            # phase 1, c-outermost over groups of 3 q/k blocks (the eps
            # psum ring depth): every input chunk c is consumed by all
            # in-flight accumulators as soon as its DMA lands, so compute
            # tracks the ~13us input stream instead of serializing on the
            # last chunk. Two v accumulators ride along per group; the
            # remaining v tiles run after the stream (inputs resident).
            v_state = {}

            def v_step(s, c):
                if s not in v_state:
                    v_state[s] = psum_pool.tile([128, 512], F32, tag="av",
                                                bufs=2, name=f"vv_{s}")[:, 0:HD]
                nc.tensor.matmul(
                    v_state[s],
                    (x_sb[:, c, s * 128:(s + 1) * 128]),
                    (w_sb[:, c, 2 * HD:3 * HD]),
                    start=(c == 0), stop=(c == EC - 1))
                if c == EC - 1:
                    nc.vector.tensor_copy(
                        v_sb[:, s, :].rearrange(
                            "p (h c) -> p h c", c=2 * D)[:, :, 0:D],
                        v_state.pop(s)[:].rearrange("p (h d) -> p h d", h=HL))

            blocks = [(which, m, nh) for which in (0, 1)
                      for m in range(MC) for nh in (0, 1)]
            for g in range(6):
                group = blocks[2 * g:2 * g + 2]
                mm4s = {}
                for b, (which, m, nh) in enumerate(group):
                    mm4s[b] = psum_pool.tile([128, 2, 512], F32, tag="eps",
                                             bufs=3, name=f"mm4_{g}_{b}")
                for c in range(EC):
                    for b, (which, m, nh) in enumerate(group):
                        lo = which * HD + m * 128
                        for j in (0, 1):
                            n = 2 * nh + j
                            nc.tensor.matmul(
                                mm4s[b][:, j, :],
                                (w_sb[:, c, lo:lo + 128]),
                                (x_sb[:, c, n * 512:(n + 1) * 512]),
                                start=(c == 0), stop=(c == EC - 1))
                    v_step(2 * g, c)
                    v_step(2 * g + 1, c)
                for b, (which, m, nh) in enumerate(group):
                    dst = qT_sb if which == 0 else kT_sb
                    for j in (0, 1):
                        n = 2 * nh + j
                        ns = slice(n * 512, (n + 1) * 512)
                        if (which + n) % 2 == 0:
                            nc.scalar.copy(dst[:, m, ns], mm4s[b][:, j, :])
                        else:
                            nc.vector.tensor_copy(dst[:, m, ns], mm4s[b][:, j, :])
            for s0 in range(12, NK, 2):
                for c in range(EC):
                    v_step(s0, c)
                    v_step(s0 + 1, c)

            esb_pool = ph2.enter_context(tc.tile_pool(name="esb", bufs=4))
            sm_pool = ph2.enter_context(tc.tile_pool(name="sm", bufs=4))
            osb_pool = ph2.enter_context(tc.tile_pool(name="osb", bufs=3))

            def make_pr_jobs(n):
                # output projection for query group n: 4 s-chunks x 2
                # halves sharing the eps psum ring; DMA fires per s-chunk
                jobs = []
                for s in range(4 * n, 4 * n + 4):
                    o_sb = osb_pool.tile([128, EMB], BF16, tag="osb",
                                         name=f"osb_{s}")
                    for half in range(2):
                        def job(s=s, half=half, o_sb=o_sb):
                            pr = psum_pool.tile([128, 2, 512], F32, tag="eps",
                                                bufs=3,
                                                name=f"pr_{s}_{half}")[:, 0, 0:HD]
                            for mm in range(MC):
                                nc.tensor.matmul(
                                    pr[:],
                                    (attT_sb[:, mm, s * 128:(s + 1) * 128]),
                                    (wp_sb[:, mm, half * HD:(half + 1) * HD]),
                                    start=(mm == 0), stop=(mm == MC - 1))
                            if half == 0:
                                nc.vector.tensor_copy(o_sb[:, 0:HD], pr[:])
                            else:
                                nc.scalar.copy(o_sb[:, HD:2 * HD], pr[:])
                                nc.sync.dma_start(outr[:, s, :], o_sb[:])
                        jobs.append(job)
                return jobs

            pr_queue = []
            pending_norm = {}
            for n in range(NQ):
                for m in range(MC):
                    unit = n * MC + m
                    ns = slice(n * 512, (n + 1) * 512)
                    kslice = lambda kk: slice(kk * 128, (kk + 1) * 128)
                    av_t = [psum_pool.tile([128, 512], F32, tag="av", bufs=2,
                                           name=f"av_{m}_{n}_{z}")
                            for z in (0, 1)]
                    mvq = []

                    def emit_av(j):
                        for z in (0, 1):
                            h = 2 * m + z
                            nc.tensor.matmul(
                                av_t[z][:],
                                (v_sb[:, j, h * 2 * D:(h + 1) * 2 * D]),
                                mvq[j][z],
                                start=(j == 0), stop=(j == NK - 1))

                    for kk in range(NK):
                        if kk >= DEPTH:
                            emit_av(kk - DEPTH)
                        e2 = psum_pool.tile([128, 2, 512], F32, tag="eps",
                                            bufs=3, name=f"e_{m}_{n}_{kk}")
                        nc.tensor.matmul(e2[:, 0, :],
                                         (kT_sb[0:64, m, kslice(kk)]),
                                         (qT_sb[0:64, m, ns]),
                                         start=True, stop=True)
                        nc.tensor.matmul(e2[:, 1, :],
                                         (kT_sb[64:128, m, kslice(kk)]),
                                         (qT_sb[64:128, m, ns]),
                                         start=True, stop=True)
                        # exp FIRST (ring-critical), then the deferred
                        # norm ops: their deps are a unit old, but emitting
                        # them earlier would queue them ahead of the exp on
                        # the same engine FIFO and delay the e2 ring.
                        if kk % 2 == 1:
                            esb = esb_pool.tile([128, 2, 512], BF16, tag="esb",
                                                bufs=7, name=f"esb_{m}_{n}_{kk}")
                            nc.scalar.activation(esb[:], e2[:], EXP, scale=SCALE)
                            mvq.append((esb[:, 0, :], esb[:, 1, :]))
                        else:
                            esf = esb_pool.tile([128, 2, 512], F32, tag="esf",
                                                bufs=7, name=f"esf_{m}_{n}_{kk}")
                            nc.vector.tensor_scalar(esf[:], e2[:], ASC, BMAGIC,
                                                    MULT, ADD)
                            bv = esf[:].bitcast(I16)[:, :, 0::2].bitcast(BF16)
                            mvq.append((bv[:, 0, :], bv[:, 1, :]))
                        if kk in pending_norm:
                            for fn in pending_norm.pop(kk):
                                fn()
                    for j in range(NK - DEPTH, NK):
                        emit_av(j)
                        if j in (11, 13, 15) and pr_queue:
                            pr_queue.pop(0)()

                    # drain + normalize for this (pair, n):
                    # rows 0:64 = av, rows 64:128 = l replicated 64-wide.
                    # copies split 3 ACT / 1 DVE; 1/l on DVE; the attT
                    # multiplies run on GPSIMD (own queue, off the exp path).
                    # All ops are spread into the next unit's first kk slots
                    # so neither engine sees a boundary burst.
                    tiles = []
                    for z in (0, 1):
                        tiles.append((
                            sm_pool.tile([D, 512], F32, tag=f"avst{z}",
                                         bufs=3, name=f"avst_{m}_{n}_{z}"),
                            sm_pool.tile([D, 512], F32, tag=f"lrep{z}",
                                         bufs=3, name=f"lrep_{m}_{n}_{z}"),
                            sm_pool.tile([D, 512], F32, tag=f"rb{z}",
                                         bufs=3, name=f"rb_{m}_{n}_{z}")))

                    def norm_ops(m=m, ns=ns, av_t=av_t, tiles=tiles):
                        a0, l0, r0 = tiles[0]
                        a1, l1, r1 = tiles[1]

                        def mul(z, a, r):
                            nc.gpsimd.tensor_mul(
                                attT_sb[z * 64:(z + 1) * 64, m, ns], a[:], r[:])
                        return {
                            1: [lambda: nc.scalar.copy(a0[:], av_t[0][0:D, :])],
                            3: [lambda: nc.scalar.copy(a1[:], av_t[1][0:D, :])],
                            5: [lambda: nc.vector.tensor_copy(
                                    l0[:], av_t[0][D:2 * D, :])],
                            7: [lambda: nc.scalar.copy(l1[:],
                                                       av_t[1][D:2 * D, :]),
                                lambda: nc.vector.reciprocal_approx_fast(
                                    r0[:], l0[:])],
                            9: [lambda: nc.vector.reciprocal_approx_fast(
                                    r1[:], l1[:]),
                                lambda: mul(0, a0, r0)],
                            11: [lambda: mul(1, a1, r1)],
                        }

                    if unit == MC * NQ - 1:
                        for kk, fns in sorted(norm_ops().items()):
                            for fn in fns:
                                fn()
                    else:
                        pending_norm = norm_ops()
                if n > 0:
                    pr_queue.extend(make_pr_jobs(n - 1))

            # the last group's output projection: job m0/m1 matmuls have
            # no deps on the terminal norm (only each m2 does), so they
            # stream immediately after the drain — no filler needed
            pr_queue.extend(make_pr_jobs(NQ - 1))
            while pr_queue:
                pr_queue.pop(0)()


_CACHE = {}


def _build():
    if "nc" not in _CACHE:
        nc = bacc.Bacc("TRN2", target_bir_lowering=False, debug=False,
                       num_devices=NCORES)
        with tile.TileContext(nc) as tc:
            _emit(tc)
        nc.compile()
        _CACHE["nc"] = nc
    return _CACHE["nc"]


def _in_maps(x, W_qkv, W_proj):
    in_maps = []
    for c in range(NCORES):
        b, g = divmod(c, 2)
        r0 = g * HD
        w_rows = np.concatenate([
            W_qkv[0 * EMB + r0: 0 * EMB + r0 + HD],
            W_qkv[1 * EMB + r0: 1 * EMB + r0 + HD],
            W_qkv[2 * EMB + r0: 2 * EMB + r0 + HD],
        ], axis=0)                                   # [1152, 768]
        bf = ml_dtypes.bfloat16
        in_maps.append({
            "xT": np.ascontiguousarray(x[b].T.astype(bf)),
            "wT": np.ascontiguousarray(w_rows.T.astype(bf)),
            "wpT": np.ascontiguousarray(W_proj[:, r0:r0 + HD].T.astype(bf)),
        })
    return in_maps


LAST_RESULTS = None


def kernel(x, W_qkv, W_proj, b_proj):
    global LAST_RESULTS
    x = np.ascontiguousarray(np.asarray(x, dtype=np.float32))
    W_qkv = np.asarray(W_qkv, dtype=np.float32)
    W_proj = np.asarray(W_proj, dtype=np.float32)
    b_proj = np.asarray(b_proj, dtype=np.float32)

    nc = _build()
    in_maps = _in_maps(x, W_qkv, W_proj)
    res = run_bass_kernel_spmd(nc, in_maps, core_ids=list(range(NCORES)))
    LAST_RESULTS = res

    out = np.empty((B, N, EMB), dtype=np.float32)
    for b in range(B):
        out[b] = (res.results[2 * b]["outp"].astype(np.float32)
                  + res.results[2 * b + 1]["outp"].astype(np.float32))
    out += b_proj
    return out

